# revision 16
# baseline (speedup 1.0000x reference)
"""AUGRU cell (attention-scaled GRU update) on 8 Trainium2 NeuronCores.

Data-parallel: batch B=65536 sharded 8 ways (8192 rows/core); gate weights
replicated.  Per core:

  gates_x = x @ W_x.T + b_x          (8192,384)
  gates_h = h @ W_h.T + b_h
  u = sigmoid(.. u block ..); r = sigmoid(.. r block ..)
  h_tilde = tanh(x_c + r * h_c)
  h_new = h_prev + att*u*(h_tilde - h_prev)

v2 design (batch on partitions, gates along free dim, bf16 matmuls):
  - x/h loaded via GPSIMD SWDGE DMA with in-flight fp32->bf16 cast.
  - per-tile [128,128] bf16 transposes via the DMA XBAR (no PE, no PSUM).
  - 3 bf16 matmuls per 128-row tile into one PSUM bank laid out
    [Cx | S_u | S_r | Ch]; a K=1 ones x bias matmul zero-fills and biases.
  - gates PSUM [128,4,512] double-buffered (8 banks total).
  - epilogue: ACT sigmoid(u|r) + tanh; DVE candidate muls + fused
    (d*att)*u; GPSIMD the two adds with h; output cast bf16->fp32 in DMA.
"""

import sys

sys.path.insert(0, "/opt/trn_rl_repo")

from contextlib import ExitStack

import numpy as np

import concourse.bass as bass
import concourse.tile as tile
from concourse import bacc, mybir
from concourse.bass_utils import run_bass_kernel_spmd
from concourse.masks import make_identity

F32 = mybir.dt.float32
BF16 = mybir.dt.bfloat16
AF = mybir.ActivationFunctionType
OP = mybir.AluOpType

import os

B = 65536
NCORES = 8
BL = int(os.environ.get("AUGRU_BL", B // NCORES))  # 8192 rows per core
I = 128
H = 128
G3 = 3 * H  # 384
P = 128
GROUP = 4  # tiles per group
ROWS = P * GROUP  # 512
NGROUPS = BL // ROWS  # 16

# PSUM bank layout per batch-tile (512 f32 = one 2KB bank):
#   [0:128] Cx' | [128:256] S_u | [256:384] S_r | [384:512] Ch'


def build_program():
    nc = bacc.Bacc("TRN2", target_bir_lowering=False, debug=False)

    x_d = nc.dram_tensor("x", [BL, I], F32, kind="ExternalInput").ap()
    h_d = nc.dram_tensor("h_prev", [BL, H], F32, kind="ExternalInput").ap()
    a_d = nc.dram_tensor("att_score", [BL], F32, kind="ExternalInput").ap()
    wx_d = nc.dram_tensor("W_x", [G3, I], F32, kind="ExternalInput").ap()
    bx_d = nc.dram_tensor("b_x", [G3], F32, kind="ExternalInput").ap()
    wh_d = nc.dram_tensor("W_h", [G3, H], F32, kind="ExternalInput").ap()
    bh_d = nc.dram_tensor("b_h", [G3], F32, kind="ExternalInput").ap()
    o_d = nc.dram_tensor("h_new", [BL, H], F32, kind="ExternalOutput").ap()

    with tile.TileContext(nc) as tc, ExitStack() as ctx:
        consts = ctx.enter_context(tc.tile_pool(name="consts", bufs=1))
        io = ctx.enter_context(tc.tile_pool(name="io", bufs=4))
        tr = ctx.enter_context(tc.tile_pool(name="tr", bufs=4))
        ep = ctx.enter_context(tc.tile_pool(name="ep", bufs=3))
        pg = ctx.enter_context(tc.tile_pool(name="pg", bufs=2, space="PSUM"))

        # ---------------- one-time setup ----------------
        ident = consts.tile([P, P], F32)
        make_identity(nc, ident)
        ones_f = consts.tile([1, P], F32, tag="ones_f")
        nc.vector.memset(ones_f, 1.0)
        ones = consts.tile([1, P], BF16)
        nc.vector.tensor_copy(ones, ones_f)
        ident_b = consts.tile([P, P], BF16, tag="ident_b")
        nc.vector.tensor_copy(ident_b, ident)

        # Weights: load [gate128, block, I], PE-transpose blocks, store bf16
        #   wtx columns [c|u|r]; wth columns [u|r|c]
        wxn = consts.tile([P, 3, I], F32, tag="wxn")
        nc.sync.dma_start(wxn, wx_d.rearrange("(b g) i -> g b i", g=P))
        whn = consts.tile([P, 3, I], F32, tag="whn")
        nc.sync.dma_start(whn, wh_d.rearrange("(b g) i -> g b i", g=P))
        wtx = consts.tile([P, G3], BF16, tag="wtx")
        wth = consts.tile([P, G3], BF16, tag="wth")
        for dst, src in ((0, 2), (1, 0), (2, 1)):
            ps = pg.tile([P, GROUP, 4 * P], F32, tag="g")
            nc.tensor.matmul(ps[:, 0, 0:P], lhsT=wxn[:, src, :], rhs=ident,
                             is_transpose=True)
            nc.vector.tensor_copy(wtx[:, dst * P : (dst + 1) * P], ps[:, 0, 0:P])
        for dst in range(3):
            ps = pg.tile([P, GROUP, 4 * P], F32, tag="g")
            nc.tensor.matmul(ps[:, 0, 0:P], lhsT=whn[:, dst, :], rhs=ident,
                             is_transpose=True)
            nc.vector.tensor_copy(wth[:, dst * P : (dst + 1) * P], ps[:, 0, 0:P])

        # att scores [128, 64]: load [64,128] then PE-transpose
        att_n = consts.tile([BL // P, P], F32, tag="attn")
        nc.sync.dma_start(att_n, a_d.rearrange("(j p) -> j p", p=P))
        att = consts.tile([P, BL // P], BF16, tag="att")
        ps = pg.tile([P, GROUP, 4 * P], F32, tag="g")
        nc.tensor.matmul(ps[:, 0, 0 : BL // P], lhsT=att_n,
                         rhs=ident[0 : BL // P, 0 : BL // P], is_transpose=True)
        nc.vector.tensor_copy(att, ps[:, 0, 0 : BL // P])
        # att broadcast along H so the blend runs all-bf16 packed ops
        attb = consts.tile([P, BL // P, H], BF16, tag="attb")
        nc.vector.tensor_copy(
            attb, att.unsqueeze(2).broadcast_to([P, BL // P, H])
        )

        # combined bias vector [b_xc | b_xu+b_hu | b_xr+b_hr | b_hc], bf16
        bxs = consts.tile([1, G3], F32, tag="bxs")
        nc.sync.dma_start(bxs, bx_d.unsqueeze(0))
        bhs = consts.tile([1, G3], F32, tag="bhs")
        nc.sync.dma_start(bhs, bh_d.unsqueeze(0))
        bias = consts.tile([1, 4 * P], BF16, tag="bias")
        nc.vector.tensor_copy(bias[:, 0:128], bxs[:, 256:384])
        nc.vector.tensor_tensor(bias[:, 128:384], bxs[:, 0:256], bhs[:, 0:256], OP.add)
        nc.vector.tensor_copy(bias[:, 384:512], bhs[:, 256:384])
        # broadcast bias to all 128 partitions (for the per-group PSUM preload)
        ps = pg.tile([P, GROUP, 4 * P], F32, tag="g")
        nc.tensor.matmul(ps[:, 0, :], lhsT=ones, rhs=bias, start=True, stop=True)
        bias_bc = consts.tile([P, 4 * P], F32, tag="bias_bc")
        nc.vector.tensor_copy(bias_bc, ps[:, 0, :])

        # ---------------- main loop (software-pipelined emission) ----------------
        # Stage A(g): loads + casts; Stage B(g): transposes + copy + matmuls;
        # Stage C(g): epilogue + store.  Emission order per iteration:
        #   A(g+1), B(g), C(g-1)  -- so PE-feeding work outranks older epilogues
        # in Tile's priority order.
        stA = [None] * (NGROUPS + 4)
        stB = [None] * (NGROUPS + 4)
        stT = [None] * (NGROUPS + 4)
        stC = [None] * (NGROUPS + 4)

        def stage_a(g):
            b0 = g * ROWS
            xn = io.tile([P, GROUP, I], F32, tag="xn")
            nc.sync.dma_start(xn, x_d[b0 : b0 + ROWS, :].rearrange("(t p) i -> p t i", p=P))
            hn = io.tile([P, GROUP, H], F32, tag="hn")
            nc.sync.dma_start(hn, h_d[b0 : b0 + ROWS, :].rearrange("(t p) i -> p t i", p=P))
            return xn, hn

        def stage_b1(g):
            xn, hn = stA[g]
            # fp32 transposes spread across all 4 banks: x_t -> bank t
            # bytes [0:512], h_t -> bank t bytes [512:1024]; x first so the
            # x copy can start mid-stream
            gp = pg.tile([P, GROUP, 4 * P], F32, tag="g")
            for t in range(GROUP):
                nc.tensor.matmul(gp[:, t, 0:P], lhsT=xn[:, t, :],
                                 rhs=ident, is_transpose=True,
                                 start=True, stop=False)
            for t in range(GROUP):
                nc.tensor.matmul(gp[:, t, P : 2 * P], lhsT=hn[:, t, :],
                                 rhs=ident, is_transpose=True,
                                 start=False, stop=True)
            return gp

        def stage_b2(g):
            gp = stB[g]
            # PSUM->SBUF copies round fp32 -> bf16 for the matmuls
            xhT = tr.tile([P, 2, GROUP, P], BF16, tag="xhT")
            nc.vector.tensor_copy(xhT[:, 0], gp[:, :, 0:P])
            nc.scalar.copy(xhT[:, 1], gp[:, :, P : 2 * P])
            return xhT

        def stage_b3(g):
            gp = stB[g]
            xhT = stT[g]
            for t in range(GROUP):
                nc.tensor.matmul(gp[:, t, :], lhsT=ones, rhs=bias,
                                 start=True, stop=False)
            for t in range(GROUP):
                nc.tensor.matmul(gp[:, t, 0:G3], lhsT=xhT[:, 0, t],
                                 rhs=wtx, start=False, stop=False)
            for t in range(GROUP):
                nc.tensor.matmul(gp[:, t, P : P + G3], lhsT=xhT[:, 1, t],
                                 rhs=wth, start=False, stop=True)

        def stage_c(g):
            b0 = g * ROWS
            xn, hn = stA[g]
            gp = stB[g]
            ur = ep.tile([P, 2, GROUP, H], BF16, tag="ur")
            nc.scalar.activation(
                ur.transpose([0, 2, 1, 3]),
                gp[:, :, 128:384].rearrange("p t (s i) -> p t s i", s=2),
                AF.Sigmoid,
            )
            m = ep.tile([P, GROUP, H], F32, tag="m")
            nc.vector.tensor_tensor(m, ur[:, 1], gp[:, :, 384:512], OP.mult)
            pre = ep.tile([P, GROUP, H], F32, tag="pre")
            nc.vector.tensor_tensor(pre, m, gp[:, :, 0:128], OP.add)
            tb = ep.tile([P, GROUP, H], BF16, tag="tb")
            nc.scalar.activation(tb, pre, AF.Tanh)
            d = ep.tile([P, GROUP, H], BF16, tag="d")
            nc.gpsimd.tensor_tensor(d, tb, hn, OP.subtract)
            e1 = ep.tile([P, GROUP, H], BF16, tag="e1")
            nc.vector.tensor_tensor(e1, d, ur[:, 0], OP.mult)
            e2 = ep.tile([P, GROUP, H], BF16, tag="e2")
            nc.vector.tensor_tensor(
                e2, e1, attb[:, g * GROUP : (g + 1) * GROUP, :], OP.mult
            )
            return e2

        def stage_c2(g):
            b0 = g * ROWS
            xn, hn = stA[g]
            e2 = stC[g]
            ho = ep.tile([P, GROUP, H], F32, tag="ho")
            nc.gpsimd.tensor_tensor(ho, e2, hn, OP.add)
            nc.sync.dma_start(
                o_d[b0 : b0 + ROWS, :].rearrange("(t p) i -> p t i", p=P), ho
            )

        for k in range(NGROUPS + 4):
            if k < NGROUPS:
                stA[k] = stage_a(k)
            if 1 <= k < NGROUPS + 1:
                g = k - 1
                stB[g] = stage_b1(g)
                stT[g] = stage_b2(g)
            if 2 <= k < NGROUPS + 2:
                stage_b3(k - 2)
            if 3 <= k < NGROUPS + 3:
                stC[k - 3] = stage_c(k - 3)
            if k >= 4:
                stage_c2(k - 4)

    nc.compile()
    return nc


_NC_CACHE = []


def _get_nc():
    if not _NC_CACHE:
        _NC_CACHE.append(build_program())
    return _NC_CACHE[0]


def kernel(x, h_prev, att_score, W_x, b_x, W_h, b_h, **_unused):
    x = np.ascontiguousarray(np.asarray(x, dtype=np.float32))
    h_prev = np.ascontiguousarray(np.asarray(h_prev, dtype=np.float32))
    att_score = np.ascontiguousarray(np.asarray(att_score, dtype=np.float32))
    W_x = np.ascontiguousarray(np.asarray(W_x, dtype=np.float32))
    b_x = np.ascontiguousarray(np.asarray(b_x, dtype=np.float32))
    W_h = np.ascontiguousarray(np.asarray(W_h, dtype=np.float32))
    b_h = np.ascontiguousarray(np.asarray(b_h, dtype=np.float32))

    nc = _get_nc()
    in_maps = []
    for c in range(NCORES):
        s = slice(c * BL, (c + 1) * BL)
        in_maps.append(
            {
                "x": np.ascontiguousarray(x[s]),
                "h_prev": np.ascontiguousarray(h_prev[s]),
                "att_score": np.ascontiguousarray(att_score[s]),
                "W_x": W_x,
                "b_x": b_x,
                "W_h": W_h,
                "b_h": b_h,
            }
        )
    res = run_bass_kernel_spmd(nc, in_maps, list(range(NCORES)))
    out = np.concatenate([res.results[c]["h_new"] for c in range(NCORES)], axis=0)
    return out


# revision 18
# speedup vs baseline: 1.3985x; 1.3985x over previous
"""AUGRU cell (attention-scaled GRU update) on 8 Trainium2 NeuronCores.

Data-parallel: batch B=65536 sharded 8 ways (8192 rows/core); gate weights
replicated.  Per core:

  gates_x = x @ W_x.T + b_x          (8192,384)
  gates_h = h @ W_h.T + b_h
  u = sigmoid(.. u block ..); r = sigmoid(.. r block ..)
  h_tilde = tanh(x_c + r * h_c)
  h_new = h_prev + att*u*(h_tilde - h_prev)

v8 design — gate-major layout, host-staged transposed operands:
  - each core receives xT/hT = x/h shard transposed to [I, rows] (a host
    layout/sharding choice; the contraction needs I on partitions either
    way) and the output is produced transposed, flipped back on the host.
  - gates live in PSUM as [gate_type][128, 512]: U/R/Cx/Ch banks.  Two
    accumulating fp32r matmuls for U and R, one each for Cx/Ch; weights
    transposed once at setup on the PE and kept fp32r (fp32-rate 1 cyc/row
    at N>=256, fp32-class accuracy).
  - biases are per-partition in this layout: sigmoid takes them via the
    ACT bias operand; the candidate path folds them into the two DVE
    scalar_tensor_tensor ops.  No bias matmuls, no device transposes,
    no PSUM round-trip copies, no casts on the matmul path.
  - epilogue: u/r/tanh outputs bf16; blend (t-h)*u*att in packed bf16 DVE
    ops; d and the final add against fp32 hT on GPSIMD keep h_prev exact.
"""

import sys

sys.path.insert(0, "/opt/trn_rl_repo")

import os
from contextlib import ExitStack

import numpy as np

import concourse.bass as bass
import concourse.tile as tile
from concourse import bacc, mybir
from concourse.bass_utils import run_bass_kernel_spmd
from concourse.masks import make_identity

F32 = mybir.dt.float32
F32R = mybir.dt.float32r
BF16 = mybir.dt.bfloat16
AF = mybir.ActivationFunctionType
OP = mybir.AluOpType

B = 65536
NCORES = 8
BL = int(os.environ.get("AUGRU_BL", B // NCORES))  # 8192 rows per core
I = 128
H = 128
G3 = 3 * H
P = 128
ROWS = 512  # batch rows per group (one fp32 PSUM bank per gate type)
NGROUPS = BL // ROWS

# PSUM banks per group: 0 = U, 1 = R, 2 = Cx, 3 = Ch   (each [128, 512])


def build_program():
    nc = bacc.Bacc("TRN2", target_bir_lowering=False, debug=False)

    xT_d = nc.dram_tensor("xT", [I, BL], F32, kind="ExternalInput").ap()
    hT_d = nc.dram_tensor("hT", [H, BL], F32, kind="ExternalInput").ap()
    a_d = nc.dram_tensor("att_score", [BL], F32, kind="ExternalInput").ap()
    wx_d = nc.dram_tensor("W_x", [G3, I], F32, kind="ExternalInput").ap()
    bx_d = nc.dram_tensor("b_x", [G3], F32, kind="ExternalInput").ap()
    wh_d = nc.dram_tensor("W_h", [G3, H], F32, kind="ExternalInput").ap()
    bh_d = nc.dram_tensor("b_h", [G3], F32, kind="ExternalInput").ap()
    o_d = nc.dram_tensor("h_newT", [H, BL], F32, kind="ExternalOutput").ap()

    with tile.TileContext(nc) as tc, ExitStack() as ctx:
        consts = ctx.enter_context(tc.tile_pool(name="consts", bufs=1))
        io = ctx.enter_context(tc.tile_pool(name="io", bufs=4))
        ep = ctx.enter_context(tc.tile_pool(name="ep", bufs=4))
        pg = ctx.enter_context(tc.tile_pool(name="pg", bufs=2, space="PSUM"))

        # ---------------- one-time setup ----------------
        ident = consts.tile([P, P], F32)
        make_identity(nc, ident)
        ones_f = consts.tile([1, P], F32, tag="ones_f")
        nc.vector.memset(ones_f, 1.0)
        ones = consts.tile([1, P], F32R, tag="ones")
        nc.vector.tensor_copy(ones, ones_f)

        # weight blocks [128 gate, 128 I] -> transposed [128 I, 128 gate],
        # rounded to fp32r for full-rate fp32 matmuls
        wxn = consts.tile([P, 3, I], F32, tag="wxn")
        nc.sync.dma_start(wxn, wx_d.rearrange("(b g) i -> g b i", g=P))
        whn = consts.tile([P, 3, I], F32, tag="whn")
        nc.sync.dma_start(whn, wh_d.rearrange("(b g) i -> g b i", g=P))
        wT = consts.tile([P, 6, P], F32R, tag="wT")  # [xu, xr, xc, hu, hr, hc]
        for j in range(3):
            ps = pg.tile([P, 4, ROWS], F32, tag="g")
            nc.tensor.matmul(ps[:, 0, 0:P], lhsT=wxn[:, j, :], rhs=ident,
                             is_transpose=True)
            nc.vector.tensor_copy(wT[:, j, :], ps[:, 0, 0:P])
        for j in range(3):
            ps = pg.tile([P, 4, ROWS], F32, tag="g")
            nc.tensor.matmul(ps[:, 0, 0:P], lhsT=whn[:, j, :], rhs=ident,
                             is_transpose=True)
            nc.vector.tensor_copy(wT[:, 3 + j, :], ps[:, 0, 0:P])

        # per-partition bias columns [128, 1]: b_u+b_hu | b_r+b_hr | b_xc | b_hc
        bxc = consts.tile([P, 3], F32, tag="bxc")
        nc.sync.dma_start(bxc, bx_d.rearrange("(b p) -> p b", p=P))
        bhc = consts.tile([P, 3], F32, tag="bhc")
        nc.sync.dma_start(bhc, bh_d.rearrange("(b p) -> p b", p=P))
        bcol = consts.tile([P, 4], F32, tag="bcol")
        nc.vector.tensor_tensor(bcol[:, 0:2], bxc[:, 0:2], bhc[:, 0:2], OP.add)
        nc.vector.tensor_copy(bcol[:, 2:3], bxc[:, 2:3])
        nc.vector.tensor_copy(bcol[:, 3:4], bhc[:, 2:3])

        # att broadcast to all partitions (bf16): attb [128, BL]
        att1 = consts.tile([1, BL], F32R, tag="att1")
        nc.sync.dma_start(att1, a_d.unsqueeze(0).bitcast(F32R))
        attb = consts.tile([P, BL], BF16, tag="attb")
        for gch in range(NGROUPS):
            ps = pg.tile([P, 4, ROWS], F32, tag="g")
            nc.tensor.matmul(
                ps[:, 0, :], lhsT=ones,
                rhs=att1[:, gch * ROWS : (gch + 1) * ROWS],
                start=True, stop=True,
            )
            nc.vector.tensor_copy(attb[:, gch * ROWS : (gch + 1) * ROWS], ps[:, 0, :])

        # ---------------- pipelined main loop ----------------
        stA = [None] * (NGROUPS + 3)
        stB = [None] * (NGROUPS + 3)
        stC = [None] * (NGROUPS + 3)

        def stage_a(g):
            b0 = g * ROWS
            xs = io.tile([P, ROWS], F32R, tag="xs")
            nc.sync.dma_start(xs, xT_d[:, b0 : b0 + ROWS].bitcast(F32R))
            hs = io.tile([P, ROWS], F32R, tag="hs")
            nc.sync.dma_start(hs, hT_d[:, b0 : b0 + ROWS].bitcast(F32R))
            return xs, hs

        def stage_b(g):
            xs, hs = stA[g]
            xr = xs
            hr = hs
            gp = pg.tile([P, 4, ROWS], F32, tag="g")
            nc.tensor.matmul(gp[:, 0, :], lhsT=wT[:, 0, :], rhs=xr, start=True, stop=False)
            nc.tensor.matmul(gp[:, 1, :], lhsT=wT[:, 1, :], rhs=xr, start=True, stop=False)
            nc.tensor.matmul(gp[:, 2, :], lhsT=wT[:, 2, :], rhs=xr, start=True, stop=True)
            nc.tensor.matmul(gp[:, 3, :], lhsT=wT[:, 5, :], rhs=hr, start=True, stop=True)
            nc.tensor.matmul(gp[:, 0, :], lhsT=wT[:, 3, :], rhs=hr, start=False, stop=True)
            nc.tensor.matmul(gp[:, 1, :], lhsT=wT[:, 4, :], rhs=hr, start=False, stop=True)
            return gp

        def stage_c(g):
            xs, hs = stA[g]
            gp = stB[g]
            u = ep.tile([P, ROWS], BF16, tag="u")
            nc.scalar.activation(u, gp[:, 0, :], AF.Sigmoid, bias=bcol[:, 0:1])
            r = ep.tile([P, ROWS], BF16, tag="r")
            nc.scalar.activation(r, gp[:, 1, :], AF.Sigmoid, bias=bcol[:, 1:2])
            m = ep.tile([P, ROWS], F32, tag="m")
            nc.vector.scalar_tensor_tensor(
                m, in0=gp[:, 3, :], scalar=bcol[:, 3:4], in1=r,
                op0=OP.add, op1=OP.mult,
            )
            pre = ep.tile([P, ROWS], F32, tag="pre")
            nc.vector.scalar_tensor_tensor(
                pre, in0=gp[:, 2, :], scalar=bcol[:, 2:3], in1=m,
                op0=OP.add, op1=OP.add,
            )
            tb = ep.tile([P, ROWS], BF16, tag="tb")
            nc.scalar.activation(tb, pre, AF.Tanh)
            d = ep.tile([P, ROWS], BF16, tag="d")
            nc.gpsimd.tensor_tensor(d, tb, hs.bitcast(F32), OP.subtract)
            e1 = ep.tile([P, ROWS], BF16, tag="e1")
            nc.vector.tensor_tensor(e1, d, u, OP.mult)
            e2 = ep.tile([P, ROWS], BF16, tag="e2")
            nc.vector.tensor_tensor(
                e2, e1, attb[:, g * ROWS : (g + 1) * ROWS], OP.mult
            )
            return e2

        def stage_c2(g):
            b0 = g * ROWS
            xs, hs = stA[g]
            e2 = stC[g]
            ho = ep.tile([P, ROWS], F32, tag="ho")
            nc.gpsimd.tensor_tensor(ho, e2, hs.bitcast(F32), OP.add)
            nc.sync.dma_start(o_d[:, b0 : b0 + ROWS], ho)

        for k in range(NGROUPS + 3):
            if k < NGROUPS:
                stA[k] = stage_a(k)
            if 1 <= k < NGROUPS + 1:
                stB[k - 1] = stage_b(k - 1)
            if 2 <= k < NGROUPS + 2:
                stC[k - 2] = stage_c(k - 2)
            if k >= 3:
                stage_c2(k - 3)

    nc.compile()
    return nc


_NC_CACHE = []


def _get_nc():
    if not _NC_CACHE:
        _NC_CACHE.append(build_program())
    return _NC_CACHE[0]


def kernel(x, h_prev, att_score, W_x, b_x, W_h, b_h, **_unused):
    x = np.asarray(x, dtype=np.float32)
    h_prev = np.asarray(h_prev, dtype=np.float32)
    att_score = np.ascontiguousarray(np.asarray(att_score, dtype=np.float32))
    W_x = np.ascontiguousarray(np.asarray(W_x, dtype=np.float32))
    b_x = np.ascontiguousarray(np.asarray(b_x, dtype=np.float32))
    W_h = np.ascontiguousarray(np.asarray(W_h, dtype=np.float32))
    b_h = np.ascontiguousarray(np.asarray(b_h, dtype=np.float32))

    nc = _get_nc()
    in_maps = []
    for c in range(NCORES):
        s = slice(c * BL, (c + 1) * BL)
        in_maps.append(
            {
                "xT": np.ascontiguousarray(x[s].T),
                "hT": np.ascontiguousarray(h_prev[s].T),
                "att_score": np.ascontiguousarray(att_score[s]),
                "W_x": W_x,
                "b_x": b_x,
                "W_h": W_h,
                "b_h": b_h,
            }
        )
    res = run_bass_kernel_spmd(nc, in_maps, list(range(NCORES)))
    out = np.concatenate(
        [np.ascontiguousarray(res.results[c]["h_newT"].T) for c in range(NCORES)],
        axis=0,
    )
    return out


# revision 19
# speedup vs baseline: 1.4412x; 1.0306x over previous
"""AUGRU cell (attention-scaled GRU update) on 8 Trainium2 NeuronCores.

Data-parallel: batch B=65536 sharded 8 ways (8192 rows/core); gate weights
replicated.  Per core:

  gates_x = x @ W_x.T + b_x          (8192,384)
  gates_h = h @ W_h.T + b_h
  u = sigmoid(.. u block ..); r = sigmoid(.. r block ..)
  h_tilde = tanh(x_c + r * h_c)
  h_new = h_prev + att*u*(h_tilde - h_prev)

v8 design — gate-major layout, host-staged transposed operands:
  - each core receives xT/hT = x/h shard transposed to [I, rows] (a host
    layout/sharding choice; the contraction needs I on partitions either
    way) and the output is produced transposed, flipped back on the host.
  - gates live in PSUM as [gate_type][128, 512]: U/R/Cx/Ch banks.  Two
    accumulating fp32r matmuls for U and R, one each for Cx/Ch; weights
    transposed once at setup on the PE and kept fp32r (fp32-rate 1 cyc/row
    at N>=256, fp32-class accuracy).
  - biases are per-partition in this layout: sigmoid takes them via the
    ACT bias operand; the candidate path folds them into the two DVE
    scalar_tensor_tensor ops.  No bias matmuls, no device transposes,
    no PSUM round-trip copies, no casts on the matmul path.
  - epilogue: u/r/tanh outputs bf16; blend (t-h)*u*att in packed bf16 DVE
    ops; d and the final add against fp32 hT on GPSIMD keep h_prev exact.
"""

import sys

sys.path.insert(0, "/opt/trn_rl_repo")

import os
from contextlib import ExitStack

import numpy as np

import concourse.bass as bass
import concourse.tile as tile
from concourse import bacc, mybir
from concourse.bass_utils import run_bass_kernel_spmd
from concourse.masks import make_identity

F32 = mybir.dt.float32
F32R = mybir.dt.float32r
BF16 = mybir.dt.bfloat16
AF = mybir.ActivationFunctionType
OP = mybir.AluOpType

B = 65536
NCORES = 8
BL = int(os.environ.get("AUGRU_BL", B // NCORES))  # 8192 rows per core
I = 128
H = 128
G3 = 3 * H
P = 128
ROWS = 512  # batch rows per group (one fp32 PSUM bank per gate type)
NGROUPS = BL // ROWS

# PSUM banks per group: 0 = U, 1 = R, 2 = Cx, 3 = Ch   (each [128, 512])


def build_program():
    nc = bacc.Bacc("TRN2", target_bir_lowering=False, debug=False)

    xT_d = nc.dram_tensor("xT", [I, BL], F32, kind="ExternalInput").ap()
    hT_d = nc.dram_tensor("hT", [H, BL], F32, kind="ExternalInput").ap()
    a_d = nc.dram_tensor("att_score", [BL], F32, kind="ExternalInput").ap()
    wx_d = nc.dram_tensor("W_x", [G3, I], F32, kind="ExternalInput").ap()
    bx_d = nc.dram_tensor("b_x", [G3], F32, kind="ExternalInput").ap()
    wh_d = nc.dram_tensor("W_h", [G3, H], F32, kind="ExternalInput").ap()
    bh_d = nc.dram_tensor("b_h", [G3], F32, kind="ExternalInput").ap()
    o_d = nc.dram_tensor("h_newT", [H, BL], F32, kind="ExternalOutput").ap()

    with tile.TileContext(nc) as tc, ExitStack() as ctx:
        consts = ctx.enter_context(tc.tile_pool(name="consts", bufs=1))
        io = ctx.enter_context(tc.tile_pool(name="io", bufs=6))
        ep = ctx.enter_context(tc.tile_pool(name="ep", bufs=4))
        pg = ctx.enter_context(tc.tile_pool(name="pg", bufs=2, space="PSUM"))

        # ---------------- one-time setup ----------------
        ident = consts.tile([P, P], F32)
        make_identity(nc, ident)
        ones_f = consts.tile([1, P], F32, tag="ones_f")
        nc.vector.memset(ones_f, 1.0)
        ones = consts.tile([1, P], F32R, tag="ones")
        nc.vector.tensor_copy(ones, ones_f)

        # weight blocks [128 gate, 128 I] -> transposed [128 I, 128 gate],
        # rounded to fp32r for full-rate fp32 matmuls
        wxn = consts.tile([P, 3, I], F32, tag="wxn")
        nc.sync.dma_start(wxn, wx_d.rearrange("(b g) i -> g b i", g=P))
        whn = consts.tile([P, 3, I], F32, tag="whn")
        nc.sync.dma_start(whn, wh_d.rearrange("(b g) i -> g b i", g=P))
        wT = consts.tile([P, 6, P], F32R, tag="wT")  # [xu, xr, xc, hu, hr, hc]
        for j in range(3):
            ps = pg.tile([P, 4, ROWS], F32, tag="g")
            nc.tensor.matmul(ps[:, 0, 0:P], lhsT=wxn[:, j, :], rhs=ident,
                             is_transpose=True)
            nc.vector.tensor_copy(wT[:, j, :], ps[:, 0, 0:P])
        for j in range(3):
            ps = pg.tile([P, 4, ROWS], F32, tag="g")
            nc.tensor.matmul(ps[:, 0, 0:P], lhsT=whn[:, j, :], rhs=ident,
                             is_transpose=True)
            nc.vector.tensor_copy(wT[:, 3 + j, :], ps[:, 0, 0:P])

        # per-partition bias columns [128, 1]: b_u+b_hu | b_r+b_hr | b_xc | b_hc
        bxc = consts.tile([P, 3], F32, tag="bxc")
        nc.sync.dma_start(bxc, bx_d.rearrange("(b p) -> p b", p=P))
        bhc = consts.tile([P, 3], F32, tag="bhc")
        nc.sync.dma_start(bhc, bh_d.rearrange("(b p) -> p b", p=P))
        bcol = consts.tile([P, 4], F32, tag="bcol")
        nc.vector.tensor_tensor(bcol[:, 0:2], bxc[:, 0:2], bhc[:, 0:2], OP.add)
        nc.vector.tensor_copy(bcol[:, 2:3], bxc[:, 2:3])
        nc.vector.tensor_copy(bcol[:, 3:4], bhc[:, 2:3])

        # att broadcast to all partitions (bf16): attb [128, BL]
        att1 = consts.tile([1, BL], F32R, tag="att1")
        nc.sync.dma_start(att1, a_d.unsqueeze(0).bitcast(F32R))
        attb = consts.tile([P, BL], BF16, tag="attb")
        for gch in range(NGROUPS):
            ps = pg.tile([P, 4, ROWS], F32, tag="g")
            nc.tensor.matmul(
                ps[:, 0, :], lhsT=ones,
                rhs=att1[:, gch * ROWS : (gch + 1) * ROWS],
                start=True, stop=True,
            )
            nc.vector.tensor_copy(attb[:, gch * ROWS : (gch + 1) * ROWS], ps[:, 0, :])

        # ---------------- pipelined main loop ----------------
        stA = [None] * (NGROUPS + 4)
        stB = [None] * (NGROUPS + 4)
        stC = [None] * (NGROUPS + 4)

        def stage_a(g):
            b0 = g * ROWS
            xs = io.tile([P, ROWS], F32R, tag="xs")
            nc.sync.dma_start(xs, xT_d[:, b0 : b0 + ROWS].bitcast(F32R))
            hs = io.tile([P, ROWS], F32R, tag="hs")
            nc.sync.dma_start(hs, hT_d[:, b0 : b0 + ROWS].bitcast(F32R))
            return xs, hs

        def stage_b(g):
            xs, hs = stA[g]
            xr = xs
            hr = hs
            gp = pg.tile([P, 4, ROWS], F32, tag="g")
            nc.tensor.matmul(gp[:, 0, :], lhsT=wT[:, 0, :], rhs=xr, start=True, stop=False)
            nc.tensor.matmul(gp[:, 1, :], lhsT=wT[:, 1, :], rhs=xr, start=True, stop=False)
            nc.tensor.matmul(gp[:, 2, :], lhsT=wT[:, 2, :], rhs=xr, start=True, stop=True)
            nc.tensor.matmul(gp[:, 3, :], lhsT=wT[:, 5, :], rhs=hr, start=True, stop=True)
            nc.tensor.matmul(gp[:, 0, :], lhsT=wT[:, 3, :], rhs=hr, start=False, stop=True)
            nc.tensor.matmul(gp[:, 1, :], lhsT=wT[:, 4, :], rhs=hr, start=False, stop=True)
            return gp

        def stage_c(g):
            xs, hs = stA[g]
            gp = stB[g]
            u = ep.tile([P, ROWS], BF16, tag="u")
            nc.scalar.activation(u, gp[:, 0, :], AF.Sigmoid, bias=bcol[:, 0:1])
            r = ep.tile([P, ROWS], BF16, tag="r")
            nc.scalar.activation(r, gp[:, 1, :], AF.Sigmoid, bias=bcol[:, 1:2])
            m = ep.tile([P, ROWS], F32, tag="m")
            nc.vector.scalar_tensor_tensor(
                m, in0=gp[:, 3, :], scalar=bcol[:, 3:4], in1=r,
                op0=OP.add, op1=OP.mult,
            )
            pre = ep.tile([P, ROWS], F32, tag="pre")
            nc.vector.scalar_tensor_tensor(
                pre, in0=gp[:, 2, :], scalar=bcol[:, 2:3], in1=m,
                op0=OP.add, op1=OP.add,
            )
            tb = ep.tile([P, ROWS], BF16, tag="tb")
            nc.scalar.activation(tb, pre, AF.Tanh)
            d = ep.tile([P, ROWS], BF16, tag="d")
            nc.gpsimd.tensor_tensor(d, tb, hs.bitcast(F32), OP.subtract)
            e1 = ep.tile([P, ROWS], BF16, tag="e1")
            nc.vector.tensor_tensor(e1, d, u, OP.mult)
            e2 = ep.tile([P, ROWS], BF16, tag="e2")
            nc.vector.tensor_tensor(
                e2, e1, attb[:, g * ROWS : (g + 1) * ROWS], OP.mult
            )
            return e2

        def stage_c2(g):
            b0 = g * ROWS
            xs, hs = stA[g]
            e2 = stC[g]
            ho = ep.tile([P, ROWS], F32, tag="ho")
            nc.gpsimd.tensor_tensor(ho, e2, hs.bitcast(F32), OP.add)
            nc.sync.dma_start(o_d[:, b0 : b0 + ROWS], ho)

        for k in range(NGROUPS + 4):
            if k < NGROUPS:
                stA[k] = stage_a(k)
            if 2 <= k < NGROUPS + 2:
                stB[k - 2] = stage_b(k - 2)
            if k >= 4:
                stage_c2(k - 4)
            if 3 <= k < NGROUPS + 3:
                stC[k - 3] = stage_c(k - 3)

    nc.compile()
    return nc


_NC_CACHE = []


def _get_nc():
    if not _NC_CACHE:
        _NC_CACHE.append(build_program())
    return _NC_CACHE[0]


def kernel(x, h_prev, att_score, W_x, b_x, W_h, b_h, **_unused):
    x = np.asarray(x, dtype=np.float32)
    h_prev = np.asarray(h_prev, dtype=np.float32)
    att_score = np.ascontiguousarray(np.asarray(att_score, dtype=np.float32))
    W_x = np.ascontiguousarray(np.asarray(W_x, dtype=np.float32))
    b_x = np.ascontiguousarray(np.asarray(b_x, dtype=np.float32))
    W_h = np.ascontiguousarray(np.asarray(W_h, dtype=np.float32))
    b_h = np.ascontiguousarray(np.asarray(b_h, dtype=np.float32))

    nc = _get_nc()
    in_maps = []
    for c in range(NCORES):
        s = slice(c * BL, (c + 1) * BL)
        in_maps.append(
            {
                "xT": np.ascontiguousarray(x[s].T),
                "hT": np.ascontiguousarray(h_prev[s].T),
                "att_score": np.ascontiguousarray(att_score[s]),
                "W_x": W_x,
                "b_x": b_x,
                "W_h": W_h,
                "b_h": b_h,
            }
        )
    res = run_bass_kernel_spmd(nc, in_maps, list(range(NCORES)))
    out = np.concatenate(
        [np.ascontiguousarray(res.results[c]["h_newT"].T) for c in range(NCORES)],
        axis=0,
    )
    return out
